# revision 7
# baseline (speedup 1.0000x reference)
"""Distributed causal-self-attention kernel for one TRN2 chip (8 NeuronCores).

Reference math (T = D = N = 4096, faithful to the oracle):
    q = x @ Wq + bq ; k = x @ Wk + bk ; v = x @ Wv + bv      # [T, D]
    scores = (q @ k.T) / sqrt(D)                             # [T, T]
    p = softmax(scores, axis=-1)
    out = p @ v.T            # i.e. out[i, j] = sum_k p[i, k] * v[j, k]

Distribution: sequence-parallel over T. Core c owns rows R_c = [512c, 512(c+1)).
Each core computes qT/kT/vT for its own rows in TRANSPOSED layout [D, 512],
all-gathers kT and vT (so every core holds full K/V), then computes its
512-row slice of the output. Compute is bf16 on the TensorEngine with fp32
PSUM accumulation (measured end-to-end rel err ~6e-3 vs the fp32 oracle).

The transposed-projection layout puts every matmul contraction on the
partition axis with zero on-chip transposes:
    scoresT tile [j,i] = kT_chunk.T @ qT_chunk   (keys j on partitions)
    E = exp(scoresT / 64)        (scores are ~N(0,1); no max-subtraction needed)
    sums[i] = sum_j E[j, i]      (matmul with a ones vector)
    out tile [i, jout] = sum_k E[k, i] * vT[k, jout], scaled by 1/sums[i]
"""

import os
import sys

import numpy as np

for _p in ("/opt/trn_rl_repo", "/root/.axon_site/_ro/trn_rl_repo"):
    if os.path.isdir(_p) and _p not in sys.path:
        sys.path.insert(0, _p)

import ml_dtypes

P = 128                 # partitions
T = 4096                # seq len == d == input feature dim
NCORES = 8
S = T // NCORES         # 512 rows owned per core
KO = T // P             # 32 contraction chunks of 128
NB = T // S             # 8 key/value blocks of 512
NSUB = S // P           # 4 row-subtiles per core
SCALE = 1.0 / 64.0      # 1/sqrt(4096)

_BF16 = ml_dtypes.bfloat16


def _build_program():
    import concourse.mybir as mybir
    from concourse import bacc
    from concourse.tile import TileContext

    f32 = mybir.dt.float32
    bf16 = mybir.dt.bfloat16
    Ident = mybir.ActivationFunctionType.Identity
    Exp = mybir.ActivationFunctionType.Exp

    nc = bacc.Bacc(
        "TRN2",
        target_bir_lowering=False,
        debug=False,
        enable_asserts=False,
        num_devices=NCORES,
    )

    # Per-core inputs. xT is x[R_c, :].T. Weights are pre-tiled on the host:
    # W_t[dt, p, ko*128 + f] = W[ko*128 + p, dt*128 + f], so the lhsT chunk
    # for output d-tile `dt`, contraction chunk `ko` is the contiguous slice
    # W_t[dt][:, ko*128:(ko+1)*128]. b3 packs the biases as
    # b3[p, t*32 + dt] = b_t[dt*128 + p] for t in (q, k, v).
    xT = nc.dram_tensor("xT", [T, S], bf16, kind="ExternalInput")
    Wq = nc.dram_tensor("Wq", [KO, P, T], bf16, kind="ExternalInput")
    Wk = nc.dram_tensor("Wk", [KO, P, T], bf16, kind="ExternalInput")
    Wv = nc.dram_tensor("Wv", [KO, P, T], bf16, kind="ExternalInput")
    b3 = nc.dram_tensor("b3", [P, 3 * KO], f32, kind="ExternalInput")
    out = nc.dram_tensor("out", [S, T], f32, kind="ExternalOutput")

    rg = [list(range(NCORES))]

    with TileContext(nc) as tc:
        with tc.tile_pool(name="dram", bufs=1, space="DRAM") as dram:
            kT_bounce = dram.tile([T, S], bf16)
            vT_bounce = dram.tile([T, S], bf16)
            # AllGather concatenates rank shards on axis 0:
            # kTg[c*T + d, r] = k[c*512 + r, d]
            kTg = dram.tile([NCORES * T, S], bf16, addr_space="Shared")
            vTg = dram.tile([NCORES * T, S], bf16, addr_space="Shared")

            with tc.tile_pool(name="persist", bufs=1) as persist:
                qT_sb = persist.tile([P, KO, S], bf16)    # qT[d, i], resident
                ones_sb = persist.tile([P, 1], f32)
                b3_sb = persist.tile([P, 3 * KO], f32)
                recip_sb = persist.tile([P, NSUB], f32)   # 1/softmax-denominator
                nc.vector.memset(ones_sb[:], 1.0)
                nc.sync.dma_start(b3_sb[:], b3[:])

                # ---------- Phase 1: projections kT, vT, qT ----------
                with tc.tile_pool(name="xTp", bufs=1) as xTp, \
                     tc.tile_pool(name="wp", bufs=8) as wp, \
                     tc.tile_pool(name="kvstage", bufs=6) as kvstage, \
                     tc.tile_pool(name="ppsum", bufs=4, space="PSUM") as ppsum:
                    xT_sb = xTp.tile([P, KO, S], bf16)
                    xr = xT[:].rearrange("(ko p) f -> p ko f", p=P)
                    for lo, hi in ((0, 1), (1, 2), (2, 4), (4, 8), (8, 16), (16, 24), (24, 32)):
                        nc.sync.dma_start(
                            xT_sb[:, lo:hi, :], xr[:, lo:hi, :])

                    # k first, then v (so their all-gathers overlap the rest
                    # of the projection compute), then q (stays in SBUF).
                    for wi, (W, bounce, boff) in enumerate((
                        (Wk, kT_bounce, KO),
                        (Wv, vT_bounce, 2 * KO),
                        (Wq, None, 0),
                    )):
                        for dt in range(KO):
                            w_sb = wp.tile([P, T], bf16, tag="w")
                            if wi == 0 and dt == 0:
                                for lo, hi in ((0, 512), (512, 1024), (1024, 2048), (2048, 4096)):
                                    nc.sync.dma_start(w_sb[:, lo:hi], W[dt][:, lo:hi])
                            else:
                                nc.sync.dma_start(w_sb[:], W[dt])
                            ps = ppsum.tile([P, S], f32, tag="pp")
                            for ko in range(KO):
                                nc.tensor.matmul(
                                    ps[:],
                                    w_sb[:, ko * P:(ko + 1) * P],
                                    xT_sb[:, ko, :],
                                    start=(ko == 0),
                                    stop=(ko == KO - 1),
                                )
                            bias = b3_sb[:, boff + dt:boff + dt + 1]
                            if bounce is None:
                                nc.scalar.activation(qT_sb[:, dt, :], ps[:], Ident, bias=bias)
                            else:
                                st = kvstage.tile([P, S], bf16, tag="st")
                                nc.scalar.activation(st[:], ps[:], Ident, bias=bias)
                                nc.sync.dma_start(bounce[dt * P:(dt + 1) * P, :], st[:])
                        if wi == 0:
                            nc.gpsimd.collective_compute(
                                "AllGather", mybir.AluOpType.bypass,
                                replica_groups=rg, ins=[kT_bounce[:]], outs=[kTg[:]],
                            )
                        elif wi == 1:
                            nc.gpsimd.collective_compute(
                                "AllGather", mybir.AluOpType.bypass,
                                replica_groups=rg, ins=[vT_bounce[:]], outs=[vTg[:]],
                            )

                # ---------- Phase 2: scoresT -> E = exp(scoresT/64) ----------
                with tc.tile_pool(name="blocks", bufs=3) as bpool, \
                     tc.tile_pool(name="Ep", bufs=1) as Ep:
                    # E_sb[p, jo, i] = exp(scores[i_global, jo*128 + p] / 64)
                    E_sb = Ep.tile([P, KO, S], bf16)
                    with tc.tile_pool(name="qkpsum", bufs=4, space="PSUM") as qkpsum:
                        for jb in range(NB):
                            kb = bpool.tile([P, KO, S], bf16, tag="blk")
                            src = kTg[jb * T:(jb + 1) * T, :].rearrange(
                                "(ko p) f -> p ko f", p=P)
                            for i4 in range(4):
                                nc.sync.dma_start(
                                    kb[:, i4 * 8:(i4 + 1) * 8, :],
                                    src[:, i4 * 8:(i4 + 1) * 8, :],
                                )
                            for js in range(NSUB):
                                ps = qkpsum.tile([P, S], f32, tag="qk")
                                for ko in range(KO):
                                    nc.tensor.matmul(
                                        ps[:],
                                        kb[:, ko, js * P:(js + 1) * P],
                                        qT_sb[:, ko, :],
                                        start=(ko == 0),
                                        stop=(ko == KO - 1),
                                    )
                                nc.scalar.activation(
                                    E_sb[:, jb * NSUB + js, :], ps[:], Exp, scale=SCALE)

                    # ---------- Phase 3: denominators + out = (E.T @ vT) / sums ----------
                    with tc.tile_pool(name="spsum", bufs=4, space="PSUM") as spsum, \
                         tc.tile_pool(name="pvpsum", bufs=4, space="PSUM") as pvpsum, \
                         tc.tile_pool(name="ostage", bufs=4) as ostage:
                        part = ostage.tile([P, S], f32, tag="part", bufs=1)
                        Ev = E_sb[:].rearrange("p ko i -> p i ko")
                        nc.vector.reduce_sum(
                            part[:], Ev, axis=mybir.AxisListType.X)
                        for ii in range(NSUB):
                            sp = spsum.tile([P, 1], f32, tag="sum")
                            nc.tensor.matmul(
                                sp[:], part[:, ii * P:(ii + 1) * P], ones_sb[:],
                                start=True, stop=True)
                            nc.vector.reciprocal(recip_sb[:, ii:ii + 1], sp[:])

                        for vb in range(NB):
                            vbt = bpool.tile([P, KO, S], bf16, tag="blk")
                            src = vTg[vb * T:(vb + 1) * T, :].rearrange(
                                "(ko p) f -> p ko f", p=P)
                            for i4 in range(4):
                                nc.sync.dma_start(
                                    vbt[:, i4 * 8:(i4 + 1) * 8, :],
                                    src[:, i4 * 8:(i4 + 1) * 8, :],
                                )
                            for ii in range(NSUB):
                                ps = pvpsum.tile([P, S], f32, tag="pv")
                                for ko in range(KO):
                                    nc.tensor.matmul(
                                        ps[:],
                                        E_sb[:, ko, ii * P:(ii + 1) * P],
                                        vbt[:, ko, :],
                                        start=(ko == 0),
                                        stop=(ko == KO - 1),
                                    )
                                ot = ostage.tile([P, S], f32, tag="ot")
                                nc.vector.tensor_scalar_mul(
                                    ot[:], ps[:], recip_sb[:, ii:ii + 1])
                                if vb == NB - 1:
                                    h = S // 2
                                    nc.sync.dma_start(
                                        out[ii * P:(ii + 1) * P, vb * S:vb * S + h],
                                        ot[:, :h])
                                    nc.sync.dma_start(
                                        out[ii * P:(ii + 1) * P, vb * S + h:(vb + 1) * S],
                                        ot[:, h:])
                                else:
                                    nc.sync.dma_start(
                                        out[ii * P:(ii + 1) * P, vb * S:(vb + 1) * S], ot[:])
    nc.compile()
    return nc


def _tile_weight(W):
    # W_t[dt, p, ko*128 + f] = W[ko*128 + p, dt*128 + f]
    W4 = np.asarray(W, dtype=np.float32).reshape(KO, P, KO, P)
    return np.ascontiguousarray(W4.transpose(2, 1, 0, 3).reshape(KO, P, T)).astype(_BF16)


def _prepare_in_maps(inputs):
    x = np.asarray(inputs["x"], dtype=np.float32)
    Wqt = _tile_weight(inputs["Wq"])
    Wkt = _tile_weight(inputs["Wk"])
    Wvt = _tile_weight(inputs["Wv"])
    b3 = np.ascontiguousarray(
        np.concatenate(
            [np.asarray(inputs[k], np.float32).reshape(KO, P).T for k in ("bq", "bk", "bv")],
            axis=1,
        )
    )
    in_maps = []
    for c in range(NCORES):
        xT_c = np.ascontiguousarray(x[c * S:(c + 1) * S, :].T).astype(_BF16)
        in_maps.append({"xT": xT_c, "Wq": Wqt, "Wk": Wkt, "Wv": Wvt, "b3": b3})
    return in_maps


def _run(inputs, trace=False, **spmd_kwargs):
    from concourse.bass_utils import run_bass_kernel_spmd

    nc = _build_program()
    in_maps = _prepare_in_maps(inputs)
    res = run_bass_kernel_spmd(
        nc, in_maps, list(range(NCORES)), trace=trace, **spmd_kwargs)
    out = np.concatenate(
        [np.asarray(res.results[c]["out"], dtype=np.float32) for c in range(NCORES)],
        axis=0,
    )
    return out, res


def kernel(**inputs):
    out, _ = _run(inputs, trace=False)
    return out


# revision 8
# speedup vs baseline: 1.0061x; 1.0061x over previous
"""Distributed causal-self-attention kernel for one TRN2 chip (8 NeuronCores).

Reference math (T = D = N = 4096, faithful to the oracle):
    q = x @ Wq + bq ; k = x @ Wk + bk ; v = x @ Wv + bv      # [T, D]
    scores = (q @ k.T) / sqrt(D)                             # [T, T]
    p = softmax(scores, axis=-1)
    out = p @ v.T            # i.e. out[i, j] = sum_k p[i, k] * v[j, k]

Distribution: sequence-parallel over T. Core c owns rows R_c = [512c, 512(c+1)).
Each core computes qT/kT/vT for its own rows in TRANSPOSED layout [D, 512],
all-gathers kT and vT (so every core holds full K/V), then computes its
512-row slice of the output. Compute is bf16 on the TensorEngine with fp32
PSUM accumulation (measured end-to-end rel err ~6e-3 vs the fp32 oracle).

The transposed-projection layout puts every matmul contraction on the
partition axis with zero on-chip transposes:
    scoresT tile [j,i] = kT_chunk.T @ qT_chunk   (keys j on partitions)
    E = exp(scoresT / 64)        (scores are ~N(0,1); no max-subtraction needed)
    sums[i] = sum_j E[j, i]      (matmul with a ones vector)
    out tile [i, jout] = sum_k E[k, i] * vT[k, jout], scaled by 1/sums[i]
"""

import os
import sys

import numpy as np

for _p in ("/opt/trn_rl_repo", "/root/.axon_site/_ro/trn_rl_repo"):
    if os.path.isdir(_p) and _p not in sys.path:
        sys.path.insert(0, _p)

import ml_dtypes

P = 128                 # partitions
T = 4096                # seq len == d == input feature dim
NCORES = 8
S = T // NCORES         # 512 rows owned per core
KO = T // P             # 32 contraction chunks of 128
NB = T // S             # 8 key/value blocks of 512
NSUB = S // P           # 4 row-subtiles per core
SCALE = 1.0 / 64.0      # 1/sqrt(4096)

_BF16 = ml_dtypes.bfloat16


def _build_program():
    import concourse.mybir as mybir
    from concourse import bacc
    from concourse.tile import TileContext

    f32 = mybir.dt.float32
    bf16 = mybir.dt.bfloat16
    Ident = mybir.ActivationFunctionType.Identity
    Exp = mybir.ActivationFunctionType.Exp

    nc = bacc.Bacc(
        "TRN2",
        target_bir_lowering=False,
        debug=False,
        enable_asserts=False,
        num_devices=NCORES,
    )

    # Per-core inputs. xT is x[R_c, :].T. Weights are pre-tiled on the host:
    # W_t[dt, p, ko*128 + f] = W[ko*128 + p, dt*128 + f], so the lhsT chunk
    # for output d-tile `dt`, contraction chunk `ko` is the contiguous slice
    # W_t[dt][:, ko*128:(ko+1)*128]. b3 packs the biases as
    # b3[p, t*32 + dt] = b_t[dt*128 + p] for t in (q, k, v).
    xT = nc.dram_tensor("xT", [T, S], bf16, kind="ExternalInput")
    Wq = nc.dram_tensor("Wq", [KO, P, T], bf16, kind="ExternalInput")
    Wk = nc.dram_tensor("Wk", [KO, P, T], bf16, kind="ExternalInput")
    Wv = nc.dram_tensor("Wv", [KO, P, T], bf16, kind="ExternalInput")
    b3 = nc.dram_tensor("b3", [P, 3 * KO], f32, kind="ExternalInput")
    out = nc.dram_tensor("out", [S, T], f32, kind="ExternalOutput")

    rg = [list(range(NCORES))]

    with TileContext(nc) as tc:
        with tc.tile_pool(name="dram", bufs=1, space="DRAM") as dram:
            kT_bounce = dram.tile([T, S], bf16)
            vT_bounce = dram.tile([T, S], bf16)
            # AllGather concatenates rank shards on axis 0:
            # kTg[c*T + d, r] = k[c*512 + r, d]
            kTg = dram.tile([NCORES * T, S], bf16, addr_space="Shared")
            vTg = dram.tile([NCORES * T, S], bf16, addr_space="Shared")

            with tc.tile_pool(name="persist", bufs=1) as persist:
                qT_sb = persist.tile([P, KO, S], bf16)    # qT[d, i], resident
                ones_sb = persist.tile([P, 1], f32)
                b3_sb = persist.tile([P, 3 * KO], f32)
                recip_sb = persist.tile([P, NSUB], f32)   # 1/softmax-denominator
                nc.vector.memset(ones_sb[:], 1.0)
                nc.sync.dma_start(b3_sb[:], b3[:])

                # ---------- Phase 1: projections kT, vT, qT ----------
                with tc.tile_pool(name="xTp", bufs=1) as xTp, \
                     tc.tile_pool(name="wp", bufs=8) as wp, \
                     tc.tile_pool(name="kvstage", bufs=6) as kvstage, \
                     tc.tile_pool(name="ppsum", bufs=4, space="PSUM") as ppsum:
                    xT_sb = xTp.tile([P, KO, S], bf16)
                    xr = xT[:].rearrange("(ko p) f -> p ko f", p=P)
                    for c4 in range(4):
                        nc.sync.dma_start(
                            xT_sb[:, 0, c4 * P:(c4 + 1) * P],
                            xr[:, 0, c4 * P:(c4 + 1) * P])
                    for lo, hi in ((1, 2), (2, 4), (4, 8), (8, 16), (16, 24), (24, 32)):
                        nc.sync.dma_start(
                            xT_sb[:, lo:hi, :], xr[:, lo:hi, :])

                    # k first, then v (so their all-gathers overlap the rest
                    # of the projection compute), then q (stays in SBUF).
                    for wi, (W, bounce, boff) in enumerate((
                        (Wk, kT_bounce, KO),
                        (Wv, vT_bounce, 2 * KO),
                        (Wq, None, 0),
                    )):
                        for dt in range(KO):
                            w_sb = wp.tile([P, T], bf16, tag="w")
                            if wi == 0 and dt == 0:
                                for lo, hi in ((0, 128), (128, 512), (512, 1024),
                                               (1024, 2048), (2048, 4096)):
                                    nc.sync.dma_start(w_sb[:, lo:hi], W[dt][:, lo:hi])
                            else:
                                nc.sync.dma_start(w_sb[:], W[dt])
                            ps = ppsum.tile([P, S], f32, tag="pp")
                            for ko in range(KO):
                                nc.tensor.matmul(
                                    ps[:],
                                    w_sb[:, ko * P:(ko + 1) * P],
                                    xT_sb[:, ko, :],
                                    start=(ko == 0),
                                    stop=(ko == KO - 1),
                                )
                            bias = b3_sb[:, boff + dt:boff + dt + 1]
                            if bounce is None:
                                nc.scalar.activation(qT_sb[:, dt, :], ps[:], Ident, bias=bias)
                            else:
                                st = kvstage.tile([P, S], bf16, tag="st")
                                nc.scalar.activation(st[:], ps[:], Ident, bias=bias)
                                nc.sync.dma_start(bounce[dt * P:(dt + 1) * P, :], st[:])
                        if wi == 0:
                            nc.gpsimd.collective_compute(
                                "AllGather", mybir.AluOpType.bypass,
                                replica_groups=rg, ins=[kT_bounce[:]], outs=[kTg[:]],
                            )
                        elif wi == 1:
                            nc.gpsimd.collective_compute(
                                "AllGather", mybir.AluOpType.bypass,
                                replica_groups=rg, ins=[vT_bounce[:]], outs=[vTg[:]],
                            )

                # ---------- Phase 2: scoresT -> E = exp(scoresT/64) ----------
                with tc.tile_pool(name="blocks", bufs=3) as bpool, \
                     tc.tile_pool(name="Ep", bufs=1) as Ep:
                    # E_sb[p, jo, i] = exp(scores[i_global, jo*128 + p] / 64)
                    E_sb = Ep.tile([P, KO, S], bf16)
                    with tc.tile_pool(name="qkpsum", bufs=4, space="PSUM") as qkpsum:
                        for jb in range(NB):
                            kb = bpool.tile([P, KO, S], bf16, tag="blk")
                            src = kTg[jb * T:(jb + 1) * T, :].rearrange(
                                "(ko p) f -> p ko f", p=P)
                            if jb == 0:
                                for lo, hi in ((0, 1), (1, 8), (8, 16), (16, 24), (24, 32)):
                                    nc.sync.dma_start(kb[:, lo:hi, :], src[:, lo:hi, :])
                            else:
                                for i4 in range(4):
                                    nc.sync.dma_start(
                                        kb[:, i4 * 8:(i4 + 1) * 8, :],
                                        src[:, i4 * 8:(i4 + 1) * 8, :],
                                    )
                            for js in range(NSUB):
                                ps = qkpsum.tile([P, S], f32, tag="qk")
                                for ko in range(KO):
                                    nc.tensor.matmul(
                                        ps[:],
                                        kb[:, ko, js * P:(js + 1) * P],
                                        qT_sb[:, ko, :],
                                        start=(ko == 0),
                                        stop=(ko == KO - 1),
                                    )
                                nc.scalar.activation(
                                    E_sb[:, jb * NSUB + js, :], ps[:], Exp, scale=SCALE)

                    # ---------- Phase 3: denominators + out = (E.T @ vT) / sums ----------
                    with tc.tile_pool(name="spsum", bufs=4, space="PSUM") as spsum, \
                         tc.tile_pool(name="pvpsum", bufs=4, space="PSUM") as pvpsum, \
                         tc.tile_pool(name="ostage", bufs=4) as ostage:
                        part = ostage.tile([P, S], f32, tag="part", bufs=1)
                        Ev = E_sb[:].rearrange("p ko i -> p i ko")
                        nc.vector.reduce_sum(
                            part[:], Ev, axis=mybir.AxisListType.X)
                        for ii in range(NSUB):
                            sp = spsum.tile([P, 1], f32, tag="sum")
                            nc.tensor.matmul(
                                sp[:], part[:, ii * P:(ii + 1) * P], ones_sb[:],
                                start=True, stop=True)
                            nc.vector.reciprocal(recip_sb[:, ii:ii + 1], sp[:])

                        for vb in range(NB):
                            vbt = bpool.tile([P, KO, S], bf16, tag="blk")
                            src = vTg[vb * T:(vb + 1) * T, :].rearrange(
                                "(ko p) f -> p ko f", p=P)
                            if vb == 0:
                                for lo, hi in ((0, 1), (1, 8), (8, 16), (16, 24), (24, 32)):
                                    nc.sync.dma_start(vbt[:, lo:hi, :], src[:, lo:hi, :])
                            else:
                                for i4 in range(4):
                                    nc.sync.dma_start(
                                        vbt[:, i4 * 8:(i4 + 1) * 8, :],
                                        src[:, i4 * 8:(i4 + 1) * 8, :],
                                    )
                            for ii in range(NSUB):
                                ps = pvpsum.tile([P, S], f32, tag="pv")
                                for ko in range(KO):
                                    nc.tensor.matmul(
                                        ps[:],
                                        E_sb[:, ko, ii * P:(ii + 1) * P],
                                        vbt[:, ko, :],
                                        start=(ko == 0),
                                        stop=(ko == KO - 1),
                                    )
                                ot = ostage.tile([P, S], f32, tag="ot")
                                nc.vector.tensor_scalar_mul(
                                    ot[:], ps[:], recip_sb[:, ii:ii + 1])
                                if vb == NB - 1:
                                    h = S // 2
                                    nc.sync.dma_start(
                                        out[ii * P:(ii + 1) * P, vb * S:vb * S + h],
                                        ot[:, :h])
                                    nc.sync.dma_start(
                                        out[ii * P:(ii + 1) * P, vb * S + h:(vb + 1) * S],
                                        ot[:, h:])
                                else:
                                    nc.sync.dma_start(
                                        out[ii * P:(ii + 1) * P, vb * S:(vb + 1) * S], ot[:])
    nc.compile()
    return nc


def _tile_weight(W):
    # W_t[dt, p, ko*128 + f] = W[ko*128 + p, dt*128 + f]
    W4 = np.asarray(W, dtype=np.float32).reshape(KO, P, KO, P)
    return np.ascontiguousarray(W4.transpose(2, 1, 0, 3).reshape(KO, P, T)).astype(_BF16)


def _prepare_in_maps(inputs):
    x = np.asarray(inputs["x"], dtype=np.float32)
    Wqt = _tile_weight(inputs["Wq"])
    Wkt = _tile_weight(inputs["Wk"])
    Wvt = _tile_weight(inputs["Wv"])
    b3 = np.ascontiguousarray(
        np.concatenate(
            [np.asarray(inputs[k], np.float32).reshape(KO, P).T for k in ("bq", "bk", "bv")],
            axis=1,
        )
    )
    in_maps = []
    for c in range(NCORES):
        xT_c = np.ascontiguousarray(x[c * S:(c + 1) * S, :].T).astype(_BF16)
        in_maps.append({"xT": xT_c, "Wq": Wqt, "Wk": Wkt, "Wv": Wvt, "b3": b3})
    return in_maps


def _run(inputs, trace=False, **spmd_kwargs):
    from concourse.bass_utils import run_bass_kernel_spmd

    nc = _build_program()
    in_maps = _prepare_in_maps(inputs)
    res = run_bass_kernel_spmd(
        nc, in_maps, list(range(NCORES)), trace=trace, **spmd_kwargs)
    out = np.concatenate(
        [np.asarray(res.results[c]["out"], dtype=np.float32) for c in range(NCORES)],
        axis=0,
    )
    return out, res


def kernel(**inputs):
    out, _ = _run(inputs, trace=False)
    return out


# revision 9
# speedup vs baseline: 1.0095x; 1.0034x over previous
"""Distributed causal-self-attention kernel for one TRN2 chip (8 NeuronCores).

Reference math (T = D = N = 4096, faithful to the oracle):
    q = x @ Wq + bq ; k = x @ Wk + bk ; v = x @ Wv + bv      # [T, D]
    scores = (q @ k.T) / sqrt(D)                             # [T, T]
    p = softmax(scores, axis=-1)
    out = p @ v.T            # i.e. out[i, j] = sum_k p[i, k] * v[j, k]

Distribution: sequence-parallel over T. Core c owns rows R_c = [512c, 512(c+1)).
Each core computes qT/kT/vT for its own rows in TRANSPOSED layout [D, 512],
all-gathers kT and vT (so every core holds full K/V), then computes its
512-row slice of the output. Compute is bf16 on the TensorEngine with fp32
PSUM accumulation (measured end-to-end rel err ~6e-3 vs the fp32 oracle).

The transposed-projection layout puts every matmul contraction on the
partition axis with zero on-chip transposes:
    scoresT tile [j,i] = kT_chunk.T @ qT_chunk   (keys j on partitions)
    E = exp(scoresT / 64)        (scores are ~N(0,1); no max-subtraction needed)
    sums[i] = sum_j E[j, i]      (matmul with a ones vector)
    out tile [i, jout] = sum_k E[k, i] * vT[k, jout], scaled by 1/sums[i]
"""

import os
import sys

import numpy as np

for _p in ("/opt/trn_rl_repo", "/root/.axon_site/_ro/trn_rl_repo"):
    if os.path.isdir(_p) and _p not in sys.path:
        sys.path.insert(0, _p)

import ml_dtypes

P = 128                 # partitions
T = 4096                # seq len == d == input feature dim
NCORES = 8
S = T // NCORES         # 512 rows owned per core
KO = T // P             # 32 contraction chunks of 128
NB = T // S             # 8 key/value blocks of 512
NSUB = S // P           # 4 row-subtiles per core
SCALE = 1.0 / 64.0      # 1/sqrt(4096)

_BF16 = ml_dtypes.bfloat16


def _build_program():
    import concourse.mybir as mybir
    from concourse import bacc
    from concourse.tile import TileContext

    f32 = mybir.dt.float32
    bf16 = mybir.dt.bfloat16
    Ident = mybir.ActivationFunctionType.Identity
    Exp = mybir.ActivationFunctionType.Exp

    nc = bacc.Bacc(
        "TRN2",
        target_bir_lowering=False,
        debug=False,
        enable_asserts=False,
        num_devices=NCORES,
    )

    # Per-core inputs. xT is x[R_c, :].T. Weights are pre-tiled on the host:
    # W_t[dt, p, ko*128 + f] = W[ko*128 + p, dt*128 + f], so the lhsT chunk
    # for output d-tile `dt`, contraction chunk `ko` is the contiguous slice
    # W_t[dt][:, ko*128:(ko+1)*128]. b3 packs the biases as
    # b3[p, t*32 + dt] = b_t[dt*128 + p] for t in (q, k, v).
    xT = nc.dram_tensor("xT", [T, S], bf16, kind="ExternalInput")
    Wq = nc.dram_tensor("Wq", [KO, P, T], bf16, kind="ExternalInput")
    Wk = nc.dram_tensor("Wk", [KO, P, T], bf16, kind="ExternalInput")
    Wv = nc.dram_tensor("Wv", [KO, P, T], bf16, kind="ExternalInput")
    b3 = nc.dram_tensor("b3", [P, 3 * KO], f32, kind="ExternalInput")
    out = nc.dram_tensor("out", [S, T], f32, kind="ExternalOutput")

    rg = [list(range(NCORES))]

    with TileContext(nc) as tc:
        with tc.tile_pool(name="dram", bufs=1, space="DRAM") as dram:
            kT_bounce = dram.tile([T, S], bf16)
            vT_bounce = dram.tile([T, S], bf16)
            # AllGather concatenates rank shards on axis 0:
            # kTg[c*T + d, r] = k[c*512 + r, d]
            kTg = dram.tile([NCORES * T, S], bf16, addr_space="Shared")
            vTg = dram.tile([NCORES * T, S], bf16, addr_space="Shared")

            with tc.tile_pool(name="persist", bufs=1) as persist:
                qT_sb = persist.tile([P, KO, S], bf16)    # qT[d, i], resident
                ones_sb = persist.tile([P, 1], f32)
                b3_sb = persist.tile([P, 3 * KO], f32)
                recip_sb = persist.tile([P, NSUB], f32)   # 1/softmax-denominator
                acc_sb = persist.tile([P, S], f32)        # per-partition partial sums of E
                nc.vector.memset(ones_sb[:], 1.0)
                nc.sync.dma_start(b3_sb[:], b3[:])

                # ---------- Phase 1: projections kT, vT, qT ----------
                with tc.tile_pool(name="xTp", bufs=1) as xTp, \
                     tc.tile_pool(name="wp", bufs=8) as wp, \
                     tc.tile_pool(name="kvstage", bufs=6) as kvstage, \
                     tc.tile_pool(name="ppsum", bufs=4, space="PSUM") as ppsum:
                    xT_sb = xTp.tile([P, KO, S], bf16)
                    xr = xT[:].rearrange("(ko p) f -> p ko f", p=P)
                    for c4 in range(4):
                        nc.sync.dma_start(
                            xT_sb[:, 0, c4 * P:(c4 + 1) * P],
                            xr[:, 0, c4 * P:(c4 + 1) * P])
                    for lo, hi in ((1, 2), (2, 4), (4, 8), (8, 16), (16, 24), (24, 32)):
                        nc.sync.dma_start(
                            xT_sb[:, lo:hi, :], xr[:, lo:hi, :])

                    # k first, then v (so their all-gathers overlap the rest
                    # of the projection compute), then q (stays in SBUF).
                    for wi, (W, bounce, boff) in enumerate((
                        (Wk, kT_bounce, KO),
                        (Wv, vT_bounce, 2 * KO),
                        (Wq, None, 0),
                    )):
                        for dt in range(KO):
                            w_sb = wp.tile([P, T], bf16, tag="w")
                            if wi == 0 and dt == 0:
                                for lo, hi in ((0, 128), (128, 512), (512, 1024),
                                               (1024, 2048), (2048, 4096)):
                                    nc.sync.dma_start(w_sb[:, lo:hi], W[dt][:, lo:hi])
                            else:
                                nc.sync.dma_start(w_sb[:], W[dt])
                            ps = ppsum.tile([P, S], f32, tag="pp")
                            for ko in range(KO):
                                nc.tensor.matmul(
                                    ps[:],
                                    w_sb[:, ko * P:(ko + 1) * P],
                                    xT_sb[:, ko, :],
                                    start=(ko == 0),
                                    stop=(ko == KO - 1),
                                )
                            bias = b3_sb[:, boff + dt:boff + dt + 1]
                            if bounce is None:
                                nc.scalar.activation(qT_sb[:, dt, :], ps[:], Ident, bias=bias)
                            else:
                                st = kvstage.tile([P, S], bf16, tag="st")
                                nc.scalar.activation(st[:], ps[:], Ident, bias=bias)
                                nc.sync.dma_start(bounce[dt * P:(dt + 1) * P, :], st[:])
                        if wi == 0:
                            nc.gpsimd.collective_compute(
                                "AllGather", mybir.AluOpType.bypass,
                                replica_groups=rg, ins=[kT_bounce[:]], outs=[kTg[:]],
                            )
                        elif wi == 1:
                            nc.gpsimd.collective_compute(
                                "AllGather", mybir.AluOpType.bypass,
                                replica_groups=rg, ins=[vT_bounce[:]], outs=[vTg[:]],
                            )

                # ---------- Phase 2: scoresT -> E = exp(scoresT/64) ----------
                with tc.tile_pool(name="blocks", bufs=3) as bpool, \
                     tc.tile_pool(name="Ep", bufs=1) as Ep:
                    # E_sb[p, jo, i] = exp(scores[i_global, jo*128 + p] / 64)
                    E_sb = Ep.tile([P, KO, S], bf16)
                    with tc.tile_pool(name="qkpsum", bufs=4, space="PSUM") as qkpsum:
                        for jb in range(NB):
                            kb = bpool.tile([P, KO, S], bf16, tag="blk")
                            src = kTg[jb * T:(jb + 1) * T, :].rearrange(
                                "(ko p) f -> p ko f", p=P)
                            if jb == 0:
                                for lo, hi in ((0, 1), (1, 8), (8, 16), (16, 24), (24, 32)):
                                    nc.sync.dma_start(kb[:, lo:hi, :], src[:, lo:hi, :])
                            else:
                                for i4 in range(4):
                                    nc.sync.dma_start(
                                        kb[:, i4 * 8:(i4 + 1) * 8, :],
                                        src[:, i4 * 8:(i4 + 1) * 8, :],
                                    )
                            for js in range(NSUB):
                                ps = qkpsum.tile([P, S], f32, tag="qk")
                                for ko in range(KO):
                                    nc.tensor.matmul(
                                        ps[:],
                                        kb[:, ko, js * P:(js + 1) * P],
                                        qT_sb[:, ko, :],
                                        start=(ko == 0),
                                        stop=(ko == KO - 1),
                                    )
                                nc.scalar.activation(
                                    E_sb[:, jb * NSUB + js, :], ps[:], Exp, scale=SCALE)
                            Evb = E_sb[:, jb * NSUB:(jb + 1) * NSUB, :].rearrange(
                                "p ko i -> p i ko")
                            if jb == 0:
                                nc.vector.reduce_sum(
                                    acc_sb[:], Evb, axis=mybir.AxisListType.X)
                            else:
                                pt = bpool.tile([P, S], f32, tag="pt", bufs=2)
                                nc.vector.reduce_sum(
                                    pt[:], Evb, axis=mybir.AxisListType.X)
                                nc.vector.tensor_add(acc_sb[:], acc_sb[:], pt[:])

                    # ---------- Phase 3: denominators + out = (E.T @ vT) / sums ----------
                    with tc.tile_pool(name="spsum", bufs=4, space="PSUM") as spsum, \
                         tc.tile_pool(name="pvpsum", bufs=4, space="PSUM") as pvpsum, \
                         tc.tile_pool(name="ostage", bufs=4) as ostage:
                        for ii in range(NSUB):
                            sp = spsum.tile([P, 1], f32, tag="sum")
                            nc.tensor.matmul(
                                sp[:], acc_sb[:, ii * P:(ii + 1) * P], ones_sb[:],
                                start=True, stop=True)
                            nc.vector.reciprocal(recip_sb[:, ii:ii + 1], sp[:])

                        for vb in range(NB):
                            vbt = bpool.tile([P, KO, S], bf16, tag="blk")
                            src = vTg[vb * T:(vb + 1) * T, :].rearrange(
                                "(ko p) f -> p ko f", p=P)
                            if vb == 0:
                                for lo, hi in ((0, 1), (1, 8), (8, 16), (16, 24), (24, 32)):
                                    nc.sync.dma_start(vbt[:, lo:hi, :], src[:, lo:hi, :])
                            else:
                                for i4 in range(4):
                                    nc.sync.dma_start(
                                        vbt[:, i4 * 8:(i4 + 1) * 8, :],
                                        src[:, i4 * 8:(i4 + 1) * 8, :],
                                    )
                            for ii in range(NSUB):
                                ps = pvpsum.tile([P, S], f32, tag="pv")
                                for ko in range(KO):
                                    nc.tensor.matmul(
                                        ps[:],
                                        E_sb[:, ko, ii * P:(ii + 1) * P],
                                        vbt[:, ko, :],
                                        start=(ko == 0),
                                        stop=(ko == KO - 1),
                                    )
                                ot = ostage.tile([P, S], f32, tag="ot")
                                nc.vector.tensor_scalar_mul(
                                    ot[:], ps[:], recip_sb[:, ii:ii + 1])
                                if vb == NB - 1:
                                    h = S // 2
                                    nc.sync.dma_start(
                                        out[ii * P:(ii + 1) * P, vb * S:vb * S + h],
                                        ot[:, :h])
                                    nc.sync.dma_start(
                                        out[ii * P:(ii + 1) * P, vb * S + h:(vb + 1) * S],
                                        ot[:, h:])
                                else:
                                    nc.sync.dma_start(
                                        out[ii * P:(ii + 1) * P, vb * S:(vb + 1) * S], ot[:])
    nc.compile()
    return nc


def _tile_weight(W):
    # W_t[dt, p, ko*128 + f] = W[ko*128 + p, dt*128 + f]
    W4 = np.asarray(W, dtype=np.float32).reshape(KO, P, KO, P)
    return np.ascontiguousarray(W4.transpose(2, 1, 0, 3).reshape(KO, P, T)).astype(_BF16)


def _prepare_in_maps(inputs):
    x = np.asarray(inputs["x"], dtype=np.float32)
    Wqt = _tile_weight(inputs["Wq"])
    Wkt = _tile_weight(inputs["Wk"])
    Wvt = _tile_weight(inputs["Wv"])
    b3 = np.ascontiguousarray(
        np.concatenate(
            [np.asarray(inputs[k], np.float32).reshape(KO, P).T for k in ("bq", "bk", "bv")],
            axis=1,
        )
    )
    in_maps = []
    for c in range(NCORES):
        xT_c = np.ascontiguousarray(x[c * S:(c + 1) * S, :].T).astype(_BF16)
        in_maps.append({"xT": xT_c, "Wq": Wqt, "Wk": Wkt, "Wv": Wvt, "b3": b3})
    return in_maps


def _run(inputs, trace=False, **spmd_kwargs):
    from concourse.bass_utils import run_bass_kernel_spmd

    nc = _build_program()
    in_maps = _prepare_in_maps(inputs)
    res = run_bass_kernel_spmd(
        nc, in_maps, list(range(NCORES)), trace=trace, **spmd_kwargs)
    out = np.concatenate(
        [np.asarray(res.results[c]["out"], dtype=np.float32) for c in range(NCORES)],
        axis=0,
    )
    return out, res


def kernel(**inputs):
    out, _ = _run(inputs, trace=False)
    return out
